# revision 3
# baseline (speedup 1.0000x reference)
"""Trainium2 Bass kernel v2 for SAGAN-style self-attention (nn_Attention_36438502539877).

Reference computation (per batch b):
    x = inputs[b].reshape(4096, 256)
    f = x @ Wf + bf; g = x @ Wg + bg; h = x @ Wh + bh     # [4096, 32]
    beta = softmax(g @ f.T, axis=-1)                       # [4096, 4096]
    out = gamma * ((beta @ h) @ Wv + bv) + x

Sharding: 8 cores = 4 batches x 2 query-halves (rows rolled so each core's
2048 queries sit at rows 0..2047). gamma folded into Wv/bv on host.

v2 design, structured around the ScalarE exp floor (~55us of exp streaming
per core at 1 elem/cycle/lane, 1.2 GHz):
  - MM1 (s^T = f^T.T @ g^T, K=32) runs 3x row-packed via tile_position
    (32x32 PE sub-array concurrency); f^T tile i lives at SBUF partitions
    32*(i%4), g^T replicated at all four partition groups.
  - MM2 (acc += h1^T @ exp(s^T), M=33) runs 2x col-packed (col groups 0/64);
    the PSUM has_written per-element semantics allow both column groups to
    share one accumulation bank (start=True only on the very first matmul).
  - Query groups of 512: a 3-tile s^T group is 3 PSUM banks; sT double-
    buffered (6 banks) + acc (1) + pro (1) = 8 banks.
  - exp consumes a whole 3-tile group per ACT instruction (free dim 1536)
    to amortize the ~313-cycle ACT instruction overhead.
  - ACT exp table pre-loaded via a dummy activation BEFORE the scalar-queue
    input DMAs; a dense PE warmup runs during the input load to flip the
    HAM clock gate to 2.4 GHz.
  - Prologue projections stage pp (PSUM) -> ppc (SBUF) with one DVE copy so
    the single PSUM bank frees immediately; f4 spreading runs on GpSimd.
  - Epilogue of query group q interleaves into q+1's m-loop; the final
    epilogue uses ACT/GpSimd/PE (all idle by then) and PE-transposes the
    denominator row instead of a DMA partition-scatter.
"""

import os
import sys

for _p in ("/opt/trn_rl_repo", "/root/.axon_site/_ro/trn_rl_repo"):
    if os.path.isdir(_p) and _p not in sys.path:
        sys.path.insert(0, _p)

import numpy as np

import concourse.bass as bass
import concourse.mybir as mybir
import concourse.tile as tile
from concourse.masks import make_identity
from concourse.vector_clock import ScopedClock

F32 = mybir.dt.float32
BF16 = mybir.dt.bfloat16

N = 4096     # keys per core (full batch image)
Q = 2048     # queries per core
C = 256      # channels
CR = 32      # attention inner dim
MTILES = N // 128   # 32 key tiles
QG = 512            # query group width
NQG = Q // QG       # 4 query groups
GS = 3              # MM1 row-packing group size
GROUPS = [tuple(range(s, min(s + GS, MTILES))) for s in range(0, MTILES, GS)]
NPAIRS = MTILES // 2  # MM2 col-packed pairs


class _TileContextSplitDrain(tile.TileContext):
    """TileContext with a post-pass splitting excess sem waits: this
    container's walrus rejects >1 sync wait on one instruction ("Too many
    sync wait commands"), so extra waits are hoisted onto standalone wait
    instructions on the same engine right before the instruction."""

    MAX_WAITS = 1

    def _split_excess_waits(self):
        import bass_rust

        nc = self.nc
        cur = nc.cur_bb.bb
        assert self.sems is not None
        id2h = {h.num: h for h in self.sems.allocated().values()}
        for f_ in nc.m.functions:
            for bb in f_.blocks:
                il = bb.instructions
                if not any(
                    inst.sync_info is not None
                    and inst.sync_info.on_wait
                    and len(inst.sync_info.on_wait) > self.MAX_WAITS
                    for inst in il
                ):
                    continue
                out = []
                for inst in il:
                    si = inst.sync_info
                    if si is not None and si.on_wait and len(si.on_wait) > self.MAX_WAITS:
                        waits = list(si.on_wait)
                        eng = nc.engines[inst.engine]
                        for w in waits[self.MAX_WAITS :]:
                            assert w.wait_mode == "sem-ge-imm", w
                            h = id2h.get(w.id) or bass_rust.SemaphoreHandle(
                                w.ant_name or f"S{w.id}", w.id
                            )
                            wi = eng.wait_ge(h, w.wait_value)
                            stolen = cur.instructions.pop()
                            assert stolen is wi.ins
                            out.append(stolen)
                        si.on_wait = waits[: self.MAX_WAITS]
                    out.append(inst)
                il[:] = out

    def _drain_and_barrier(self, tick_clock, wait_clock):
        nc = self.nc
        drain_inst = nc.sync.drain()
        wait_clock.add_sem_waits(
            drain_inst.ins, ScopedClock({None: tick_clock.global_clock})
        )
        self._split_excess_waits()
        nc.all_engine_barrier(sem_only=True)
        popped = nc._tile_sem_poison_stack.pop()
        assert popped is self._sem_poison
        assert self.sems is not None
        nc.clear_and_free_semaphores(list(self.sems.allocated().values()))
        nc.all_engine_barrier(sem_only=True)


def build_program():
    nc = bass.Bass("TRN2", target_bir_lowering=False, debug=False)

    x_d = nc.dram_tensor("x", [Q, C], F32, kind="ExternalInput").ap()
    xt_d = nc.dram_tensor("xt", [2, 128, N], BF16, kind="ExternalInput").ap()
    wfgh_d = nc.dram_tensor("wfgh", [128, 2, 96], BF16, kind="ExternalInput").ap()
    bias4_d = nc.dram_tensor("bias4", [128, 3], F32, kind="ExternalInput").ap()
    wva2_d = nc.dram_tensor("wva2", [128, 128], BF16, kind="ExternalInput").ap()
    out_d = nc.dram_tensor("out", [Q, C], F32, kind="ExternalOutput").ap()

    exp_fn = mybir.ActivationFunctionType.Exp
    copy_fn = mybir.ActivationFunctionType.Copy

    with _TileContextSplitDrain(nc) as tc:
        with (
            nc.allow_low_precision(reason="bf16 attention-path matmuls"),
            tc.tile_pool(name="singles", bufs=1) as singles,
            tc.tile_pool(name="expp", bufs=3) as expp,
            tc.tile_pool(name="small", bufs=2) as small,
            tc.tile_pool(name="outp", bufs=3) as outp,
            tc.tile_pool(name="ps_sT", bufs=2, space="PSUM") as ps_sT,
            tc.tile_pool(name="ps_acc", bufs=1, space="PSUM") as ps_acc,
            tc.tile_pool(name="ps_pro", bufs=1, space="PSUM") as ps_pro,
        ):
            # --- static SBUF tensors ---
            x_sb = singles.tile([128, Q // 128, C], F32, name="x_sb")
            xTq = [
                [singles.tile([128, 1024], BF16, name=f"xT{j}_{q}") for q in range(4)]
                for j in range(2)
            ]
            # f4: f^T tile i at partitions 32*(i%4), slot i//4
            f4 = singles.tile([128, MTILES // 4, 128], BF16, name="f4")
            # g4: g^T replicated at partition groups 0/32/64/96
            g4 = singles.tile([128, Q], BF16, name="g4")
            h_sb = singles.tile([CR, N], BF16, name="h_sb")
            h1_sb = singles.tile([128, MTILES, 33], BF16, name="h1_sb")
            wfgh_sb = singles.tile([128, 2, 96], BF16, name="wfgh_sb")
            bias4_sb = singles.tile([128, 3], F32, name="bias4_sb")
            wva2_sb = singles.tile([128, 128], BF16, name="wva2_sb")
            identb = singles.tile([32, 32], BF16, name="identb")
            identb128 = singles.tile([128, 128], BF16, name="identb128")
            ones128 = singles.tile([128, MTILES], F32, name="ones128")
            ones1b = singles.tile([33, 1], BF16, name="ones1b")
            dummy1 = singles.tile([1, 2], F32, name="dummy1")

            # --- local init first: the dummy activation makes walrus put
            # the ACT exp-table load at the head of the scalar queue,
            # before that queue's input DMAs ---
            nc.vector.memset(dummy1[:], 0.0)
            nc.scalar.activation(dummy1[:, 1:2], dummy1[:, 0:1], exp_fn)
            make_identity(nc, identb[:])
            make_identity(nc, identb128[:])
            nc.vector.memset(ones128[:], 1.0)
            nc.vector.memset(ones1b[:], 1.0)
            nc.vector.tensor_copy(h1_sb[:, :, 32], ones128[:])

            # --- input loads spread across the three DMA-capable queues ---
            nc.sync.dma_start(out=wfgh_sb[:], in_=wfgh_d)
            nc.scalar.dma_start(out=bias4_sb[:], in_=bias4_d)
            nc.sync.dma_start(out=xTq[0][0][:], in_=xt_d[0, :, 0:1024])
            nc.scalar.dma_start(out=xTq[1][0][:], in_=xt_d[1, :, 0:1024])
            nc.sync.dma_start(out=xTq[0][1][:], in_=xt_d[0, :, 1024:2048])
            nc.scalar.dma_start(out=xTq[1][1][:], in_=xt_d[1, :, 1024:2048])
            nc.sync.dma_start(out=xTq[0][2][:], in_=xt_d[0, :, 2048:3072])
            nc.gpsimd.dma_start(out=xTq[1][2][:], in_=xt_d[1, :, 2048:3072])
            nc.gpsimd.dma_start(out=xTq[0][3][:], in_=xt_d[0, :, 3072:4096])
            nc.gpsimd.dma_start(out=xTq[1][3][:], in_=xt_d[1, :, 3072:4096])
            nc.sync.dma_start(out=wva2_sb[:], in_=wva2_d)

            def produce_chunk(c):
                """f4/g4/h1 tiles for key rows [512c, 512c+512)."""
                nsl = slice(512 * c, 512 * (c + 1))
                pp = ps_pro.tile([96, 512], F32, tag="pro", name=f"pp{c}")
                qsl4 = slice(512 * (c % 2), 512 * (c % 2) + 512)
                nc.tensor.matmul(
                    pp[:], wfgh_sb[:, 0, :], xTq[0][c // 2][:, qsl4],
                    start=True, stop=False,
                )
                nc.tensor.matmul(
                    pp[:], wfgh_sb[:, 1, :], xTq[1][c // 2][:, qsl4],
                    start=False, stop=True,
                )
                # single staging copy frees the PSUM bank immediately.
                # chunks 0/1 skip it (next pp waits on its xt DMA anyway)
                # to keep the ~0.7us CAST off the startup critical path.
                if c >= 2:
                    ppc = small.tile([96, 512], BF16, tag="ppc", name=f"ppc{c}")
                    nc.vector.tensor_copy(ppc[:], pp[:])
                else:
                    ppc = pp
                for j in range(4):
                    psl = slice(32 * j, 32 * j + 32)
                    nc.vector.tensor_scalar_add(
                        f4[psl, c, :], ppc[0:32, 128 * j : 128 * j + 128],
                        bias4_sb[psl, 0:1],
                    )
                if c < 4:  # g only needed for this core's 2048 queries
                    nc.vector.tensor_scalar_add(
                        g4[0:32, nsl], ppc[32:64, :], bias4_sb[0:32, 1:2]
                    )
                    # replicate this query-chunk to partition groups 1-3.
                    # chunk 0 gates qg0's m-loop start: replicate on DVE
                    # (no DMA latency). Later chunks are needed only when
                    # their query group starts (~17us+ later): SBUF->SBUF
                    # DMAs on sync/gpsimd (NOT scalar: a blocked scalar
                    # dispatch would head-of-line block the exps).
                    if c == 0:
                        for j in range(1, 4):
                            nc.vector.tensor_copy(
                                g4[32 * j : 32 * j + 32, nsl], g4[0:32, nsl]
                            )
                    else:
                        nc.sync.dma_start(out=g4[32:64, nsl], in_=g4[0:32, nsl])
                        nc.sync.dma_start(out=g4[64:96, nsl], in_=g4[0:32, nsl])
                        nc.gpsimd.dma_start(out=g4[96:128, nsl], in_=g4[0:32, nsl])
                nc.vector.tensor_scalar_add(
                    h_sb[:, nsl], ppc[64:96, :], bias4_sb[0:32, 2:3]
                )
                # h1 tiles 4c..4c+3: h^T [32, 128] -> [128, 32], ones col stays
                ph = ps_pro.tile([128, 128], BF16, tag="pro", name=f"ph{c}")
                for k in range(4):
                    i = 4 * c + k
                    nc.tensor.matmul(
                        ph[:, 32 * k : 32 * k + 32],
                        h_sb[:, 128 * i : 128 * i + 128],
                        identb[:],
                        is_transpose=True,
                    )
                nc.vector.tensor_copy(h1_sb[:, 4 * c : 4 * c + 4, 0:32], ph[:])

            # --- prologue: first two chunks only; chunks 2-7 interleave
            # into qg0's m-loop (each emitted >=2 groups before its f4
            # consumer) so the PE queue starts attention work early ---
            PSCHED = {0: [2], 2: [3], 3: [4], 4: [5], 6: [6], 7: [7]}
            for c in (0, 1):
                produce_chunk(c)
            # residual x, needed only at the epilogues (t >= ~30us)
            x_view = x_d.rearrange("(t p) c -> p t c", p=128)
            for gq in range(2):
                nc.gpsimd.dma_start(
                    out=x_sb[:, 8 * gq : 8 * gq + 8, :],
                    in_=x_view[:, 8 * gq : 8 * gq + 8, :],
                )

            # --- m-loop machinery ---
            def mm1_group(qg, gi):
                grp = GROUPS[gi]
                qsl = slice(QG * qg, QG * qg + QG)
                sT = ps_sT.tile(
                    [128, len(grp), QG], F32, tag="sT", name=f"sT{qg}_{gi}"
                )
                for k, i in enumerate(grp):
                    j = i % 4
                    psl = slice(32 * j, 32 * j + 32)
                    nc.tensor.matmul(
                        sT[:, k, :],
                        f4[psl, i // 4, :],
                        g4[psl, qsl],
                        tile_position=(32 * j, 0),
                    )
                expt = expp.tile(
                    [128, len(grp), QG], BF16, tag="expt", name=f"expt{qg}_{gi}"
                )
                nc.scalar.activation(expt[:], sT[:], exp_fn)
                return expt

            def mm2_pair(qg, p, exptiles, acc):
                # acc is pre-zeroed by a DVE memset; start=False everywhere
                # makes the per-element has_written semantics correct no
                # matter how the bits are staled across the two col groups
                # (accumulate-onto-0 and overwrite are then equivalent).
                for col, i in enumerate((2 * p, 2 * p + 1)):
                    et, slot = exptiles[i]
                    base = 64 * col
                    nc.tensor.matmul(
                        acc[base : base + 33, :],
                        h1_sb[:, i, :],
                        et[:, slot, :],
                        start=False,
                        stop=(p == NPAIRS - 1 and col == 1),
                        skip_group_check=True,
                    )

            def epilogue_steps(qg, acc):
                """Yields (kind, fn) steps; kind 'pe' steps cost PE time.
                tail=True spreads work over ACT/GpSimd (idle in the tail)."""
                tail = qg == NQG - 1
                vv2 = small.tile([128, QG], BF16, tag="vv", name=f"vv{qg}")
                vhi = small.tile([33, QG], BF16, tag="vhi", name=f"vhi{qg}")
                rcp = small.tile([128, QG // 128], F32, tag="rcp", name=f"rcp{qg}")
                oT_sb = small.tile([128, 2, QG], BF16, tag="oTsb", name=f"oTsb{qg}")

                def s_vv():
                    if tail:  # split halves across ACT and DVE
                        nc.scalar.activation(vhi[:, 0:256], acc[64:97, 0:256], copy_fn)
                        nc.vector.tensor_copy(vhi[:, 256:512], acc[64:97, 256:512])
                    else:
                        nc.vector.tensor_copy(vhi[:], acc[64:97, :])
                    nc.vector.tensor_add(vv2[0:33, :], acc[0:33, :], vhi[:])
                    nc.vector.tensor_copy(vv2[64:97, :], vv2[0:33, :])

                def s_oT():
                    oT_ps = ps_sT.tile([128, 2, QG], F32, tag="sT", name=f"oT{qg}")
                    nc.tensor.matmul(oT_ps[:, 0, :], wva2_sb[0:33, :], vv2[0:33, :])
                    nc.tensor.matmul(oT_ps[:, 1, :], wva2_sb[64:97, :], vv2[64:97, :])
                    # denominator row -> per-partition layout via PE transpose
                    # bf16 PSUM writes must be 4B aligned: use stride-2 cols
                    dn_ps = ps_pro.tile(
                        [128, QG // 128, 2], BF16, tag="pro", name=f"dn{qg}"
                    )
                    for t in range(QG // 128):
                        nc.tensor.matmul(
                            dn_ps[:, t, 0:1],
                            vv2[32:33, 128 * t : 128 * t + 128],
                            ones1b[32:33, :],
                            is_transpose=True,
                        )
                    nc.vector.reciprocal(rcp[:], dn_ps[:, :, 0])
                    if tail:  # split ACT/DVE so po(t=0) starts early
                        nc.scalar.activation(
                            oT_sb[:, :, 0:256], oT_ps[:, :, 0:256], copy_fn
                        )
                        nc.vector.tensor_copy(
                            oT_sb[:, :, 256:512], oT_ps[:, :, 256:512]
                        )
                    else:
                        nc.vector.tensor_copy(oT_sb[:], oT_ps[:])

                yield "dve", s_vv
                yield "pe", s_oT

                def make_t(t):
                    def s_t():
                        # tail: po double-buffers in the (free) sT ring
                        po = (ps_sT if tail else ps_pro).tile(
                            [128, C], BF16, tag="sT" if tail else "pro",
                            name=f"po{qg}_{t}",
                        )
                        for half in range(2):
                            nc.tensor.matmul(
                                po[:, 128 * half : 128 * half + 128],
                                oT_sb[:, half, 128 * t : 128 * t + 128],
                                identb128[:],
                                is_transpose=True,
                            )
                        om = outp.tile([128, C], F32, tag="om", name=f"om{qg}_{t}")
                        outt = outp.tile([128, C], F32, tag="outt", name=f"outt{qg}_{t}")
                        if tail:
                            nc.scalar.activation(
                                om[:], po[:], copy_fn, scale=rcp[:, t : t + 1]
                            )
                        else:
                            nc.vector.tensor_scalar_mul(om[:], po[:], rcp[:, t : t + 1])
                        nc.vector.tensor_add(
                            outt[:], om[:], x_sb[:, (QG // 128) * qg + t, :]
                        )
                        row0 = QG * qg + 128 * t
                        queue = nc.scalar if (tail and t % 2) else nc.sync
                        queue.dma_start(out=out_d[row0 : row0 + 128, :], in_=outt[:])
                    return s_t

                for t in range(QG // 128):
                    yield "pe", make_t(t)

            # --- query-group loops with epilogue interleaving. The last
            # MM2 pairs of a query group are deferred until after the next
            # group's first MM1 so they don't head-block the PE queue
            # while waiting on the final exp. ---
            prev_epilogue = None  # generator of previous qg's epilogue steps
            carry_pairs = None    # deferred last pairs of the previous qg
            for qg in range(NQG):
                acc = None  # allocated after the previous qg's carried pairs
                exptiles = []
                next_pair = 0

                def drain_epilogue(n_pe):
                    nonlocal prev_epilogue
                    if prev_epilogue is None:
                        return
                    done = 0
                    for kind, fn in prev_epilogue:
                        fn()
                        if kind == "pe":
                            done += 1
                            if done >= n_pe:
                                return
                    prev_epilogue = None

                for gi, grp in enumerate(GROUPS):
                    if qg == 0:
                        for c in PSCHED.get(gi, ()):
                            produce_chunk(c)
                    expt = mm1_group(qg, gi)
                    for k in range(len(grp)):
                        exptiles.append((expt, k))
                    if gi == 1:
                        # carried pairs wait on the previous qg's last exp,
                        # which ran during this qg's first two MM1 groups
                        if carry_pairs is not None:
                            carry_pairs()
                            carry_pairs = None
                        # the previous epilogue's acc reads (s_vv) MUST be
                        # emitted before this qg's acc takes over the bank
                        if prev_epilogue is not None:
                            kind, fn = next(prev_epilogue)
                            assert kind == "dve"
                            fn()
                        acc = ps_acc.tile(
                            [128, QG], F32, tag="acc", name=f"acc{qg}"
                        )
                        nc.vector.memset(acc[:], 0.0)
                    # one epilogue PE step per group, starting once the
                    # carried-pair/epilogue dependency chain has cleared
                    if gi >= 3:
                        drain_epilogue(1)
                    # emit MM2 pairs whose tiles are in groups <= gi-1
                    if gi >= 2:
                        hi = 3 * gi - 1  # last tile index of group gi-1
                        while next_pair < NPAIRS and 2 * next_pair + 1 <= hi:
                            mm2_pair(qg, next_pair, exptiles, acc)
                            next_pair += 1
                drain_epilogue(99)
                if qg < NQG - 1:
                    def make_carry(qg_, tiles_, start_, acc_):
                        def emit():
                            p = start_
                            while p < NPAIRS:
                                mm2_pair(qg_, p, tiles_, acc_)
                                p += 1
                        return emit
                    carry_pairs = make_carry(qg, list(exptiles), next_pair, acc)
                else:
                    while next_pair < NPAIRS:
                        mm2_pair(qg, next_pair, exptiles, acc)
                        next_pair += 1
                prev_epilogue = epilogue_steps(qg, acc)
            # final epilogue (tail)
            for kind, fn in prev_epilogue:
                fn()

    return nc


_NC = None


def _get_nc():
    global _NC
    if _NC is None:
        _NC = build_program()
    return _NC


def _host_prep(inputs, Wf, bf, Wg, bg, Wh, bh, Wv, bv, gamma):
    import ml_dtypes

    x = np.asarray(inputs, np.float32).reshape(4, N, C)
    wfgh = np.concatenate(
        [np.asarray(Wf, np.float32), np.asarray(Wg, np.float32), np.asarray(Wh, np.float32)],
        axis=1,
    ).astype(ml_dtypes.bfloat16)  # [256, 96]
    # pre-arranged for a contiguous [128, 2, 96] load: [p, j, :] = row 128j+p
    wfgh = np.ascontiguousarray(wfgh.reshape(2, 128, 96).transpose(1, 0, 2))
    bias4 = np.zeros((128, 3), np.float32)
    bias4[:, 0] = np.tile(np.asarray(bf, np.float32), 4)
    bias4[:, 1] = np.tile(np.asarray(bg, np.float32), 4)
    bias4[0:32, 2] = np.asarray(bh, np.float32)
    gma = np.float32(np.asarray(gamma).reshape(-1)[0])
    wva = np.concatenate(
        [np.asarray(Wv, np.float32) * gma, (np.asarray(bv, np.float32) * gma)[None, :]],
        axis=0,
    )  # [33, 256] = [gamma*Wv; gamma*bv]
    wva2 = np.zeros((128, 128), np.float32)
    wva2[0:33, :] = wva[:, 0:128]
    wva2[64:97, :] = wva[:, 128:256]
    wva2 = wva2.astype(ml_dtypes.bfloat16)
    in_maps = []
    for core in range(8):
        b, qh = divmod(core, 2)
        xb = x[b]
        if qh:
            xb = np.roll(xb, -qh * Q, axis=0)
        xt = np.ascontiguousarray(
            xb.T.reshape(2, 128, N).astype(ml_dtypes.bfloat16)
        )  # [2, 128, N]: xt[j, p, n] = xb[n, 128j+p]
        in_maps.append(
            {
                "x": np.ascontiguousarray(xb[:Q]),
                "xt": xt,
                "wfgh": wfgh,
                "bias4": bias4,
                "wva2": wva2,
            }
        )
    return in_maps


def _gather(results, inputs_shape, dtype):
    out = np.empty((4, N, C), np.float32)
    for core in range(8):
        b, qh = divmod(core, 2)
        out[b, qh * Q : (qh + 1) * Q, :] = results[core]["out"]
    return out.reshape(inputs_shape).astype(dtype, copy=False)


def kernel(**inputs):
    from concourse.bass_utils import run_bass_kernel_spmd

    in_maps = _host_prep(**inputs)
    nc = _get_nc()
    res = run_bass_kernel_spmd(nc, in_maps, list(range(8)))
    x_in = np.asarray(inputs["inputs"])
    return _gather(res.results, x_in.shape, x_in.dtype)


def kernel_profiled(**inputs):
    """Like kernel() but with NTFF tracing; returns (out, BassKernelResults)."""
    import types

    if "antenv.axon_hooks" not in sys.modules:
        mod = types.ModuleType("antenv.axon_hooks")
        mod._h = None
        mod.set_axon_ntff_profile_hook = lambda h: setattr(mod, "_h", h)
        mod.get_axon_ntff_profile_hook = lambda: mod._h
        sys.modules["antenv.axon_hooks"] = mod
        try:
            from trn_agent_boot.trn_boot import _ntff_profile_via_ctypes

            mod._h = _ntff_profile_via_ctypes("/opt/axon/libaxon_pjrt.so")
        except Exception as e:  # profiling unavailable; run untraced
            print("NTFF hook unavailable:", e)
    from concourse.bass_utils import run_bass_kernel_spmd

    in_maps = _host_prep(**inputs)
    nc = _get_nc()
    res = run_bass_kernel_spmd(nc, in_maps, list(range(8)), trace=True)
    x_in = np.asarray(inputs["inputs"])
    return _gather(res.results, x_in.shape, x_in.dtype), res


# revision 4
# speedup vs baseline: 1.0153x; 1.0153x over previous
"""Trainium2 Bass kernel v2 for SAGAN-style self-attention (nn_Attention_36438502539877).

Reference computation (per batch b):
    x = inputs[b].reshape(4096, 256)
    f = x @ Wf + bf; g = x @ Wg + bg; h = x @ Wh + bh     # [4096, 32]
    beta = softmax(g @ f.T, axis=-1)                       # [4096, 4096]
    out = gamma * ((beta @ h) @ Wv + bv) + x

Sharding: 8 cores = 4 batches x 2 query-halves (rows rolled so each core's
2048 queries sit at rows 0..2047). gamma folded into Wv/bv on host.

v2 design, structured around the ScalarE exp floor (~55us of exp streaming
per core at 1 elem/cycle/lane, 1.2 GHz):
  - MM1 (s^T = f^T.T @ g^T, K=32) runs 3x row-packed via tile_position
    (32x32 PE sub-array concurrency); f^T tile i lives at SBUF partitions
    32*(i%4), g^T replicated at all four partition groups.
  - MM2 (acc += h1^T @ exp(s^T), M=33) runs 2x col-packed (col groups 0/64);
    the PSUM has_written per-element semantics allow both column groups to
    share one accumulation bank (start=True only on the very first matmul).
  - Query groups of 512: a 3-tile s^T group is 3 PSUM banks; sT double-
    buffered (6 banks) + acc (1) + pro (1) = 8 banks.
  - exp consumes a whole 3-tile group per ACT instruction (free dim 1536)
    to amortize the ~313-cycle ACT instruction overhead.
  - ACT exp table pre-loaded via a dummy activation BEFORE the scalar-queue
    input DMAs; a dense PE warmup runs during the input load to flip the
    HAM clock gate to 2.4 GHz.
  - Prologue projections stage pp (PSUM) -> ppc (SBUF) with one DVE copy so
    the single PSUM bank frees immediately; f4 spreading runs on GpSimd.
  - Epilogue of query group q interleaves into q+1's m-loop; the final
    epilogue uses ACT/GpSimd/PE (all idle by then) and PE-transposes the
    denominator row instead of a DMA partition-scatter.
"""

import os
import sys

for _p in ("/opt/trn_rl_repo", "/root/.axon_site/_ro/trn_rl_repo"):
    if os.path.isdir(_p) and _p not in sys.path:
        sys.path.insert(0, _p)

import numpy as np

import concourse.bass as bass
import concourse.mybir as mybir
import concourse.tile as tile
from concourse.masks import make_identity
from concourse.vector_clock import ScopedClock

F32 = mybir.dt.float32
BF16 = mybir.dt.bfloat16

N = 4096     # keys per core (full batch image)
Q = 2048     # queries per core
C = 256      # channels
CR = 32      # attention inner dim
MTILES = N // 128   # 32 key tiles
QG = 512            # query group width
NQG = Q // QG       # 4 query groups
GS = 3              # MM1 row-packing group size
GROUPS = [tuple(range(s, min(s + GS, MTILES))) for s in range(0, MTILES, GS)]
NPAIRS = MTILES // 2  # MM2 col-packed pairs


class _TileContextSplitDrain(tile.TileContext):
    """TileContext with a post-pass splitting excess sem waits: this
    container's walrus rejects >1 sync wait on one instruction ("Too many
    sync wait commands"), so extra waits are hoisted onto standalone wait
    instructions on the same engine right before the instruction."""

    MAX_WAITS = 1

    def _split_excess_waits(self):
        import bass_rust

        nc = self.nc
        cur = nc.cur_bb.bb
        assert self.sems is not None
        id2h = {h.num: h for h in self.sems.allocated().values()}
        for f_ in nc.m.functions:
            for bb in f_.blocks:
                il = bb.instructions
                if not any(
                    inst.sync_info is not None
                    and inst.sync_info.on_wait
                    and len(inst.sync_info.on_wait) > self.MAX_WAITS
                    for inst in il
                ):
                    continue
                out = []
                for inst in il:
                    si = inst.sync_info
                    if si is not None and si.on_wait and len(si.on_wait) > self.MAX_WAITS:
                        waits = list(si.on_wait)
                        eng = nc.engines[inst.engine]
                        for w in waits[self.MAX_WAITS :]:
                            assert w.wait_mode == "sem-ge-imm", w
                            h = id2h.get(w.id) or bass_rust.SemaphoreHandle(
                                w.ant_name or f"S{w.id}", w.id
                            )
                            wi = eng.wait_ge(h, w.wait_value)
                            stolen = cur.instructions.pop()
                            assert stolen is wi.ins
                            out.append(stolen)
                        si.on_wait = waits[: self.MAX_WAITS]
                    out.append(inst)
                il[:] = out

    def _drain_and_barrier(self, tick_clock, wait_clock):
        nc = self.nc
        drain_inst = nc.sync.drain()
        wait_clock.add_sem_waits(
            drain_inst.ins, ScopedClock({None: tick_clock.global_clock})
        )
        self._split_excess_waits()
        nc.all_engine_barrier(sem_only=True)
        popped = nc._tile_sem_poison_stack.pop()
        assert popped is self._sem_poison
        assert self.sems is not None
        nc.clear_and_free_semaphores(list(self.sems.allocated().values()))
        nc.all_engine_barrier(sem_only=True)


def build_program():
    nc = bass.Bass("TRN2", target_bir_lowering=False, debug=False)

    x_d = nc.dram_tensor("x", [Q, C], F32, kind="ExternalInput").ap()
    xt_d = nc.dram_tensor("xt", [2, 128, N], BF16, kind="ExternalInput").ap()
    wfgh_d = nc.dram_tensor("wfgh", [128, 2, 96], BF16, kind="ExternalInput").ap()
    bias4_d = nc.dram_tensor("bias4", [128, 3], F32, kind="ExternalInput").ap()
    wva2_d = nc.dram_tensor("wva2", [128, 128], BF16, kind="ExternalInput").ap()
    out_d = nc.dram_tensor("out", [Q, C], F32, kind="ExternalOutput").ap()

    exp_fn = mybir.ActivationFunctionType.Exp
    copy_fn = mybir.ActivationFunctionType.Copy

    with _TileContextSplitDrain(nc) as tc:
        with (
            nc.allow_low_precision(reason="bf16 attention-path matmuls"),
            tc.tile_pool(name="singles", bufs=1) as singles,
            tc.tile_pool(name="expp", bufs=3) as expp,
            tc.tile_pool(name="small", bufs=2) as small,
            tc.tile_pool(name="outp", bufs=3) as outp,
            tc.tile_pool(name="ps_sT", bufs=2, space="PSUM") as ps_sT,
            tc.tile_pool(name="ps_acc", bufs=1, space="PSUM") as ps_acc,
            tc.tile_pool(name="ps_pro", bufs=1, space="PSUM") as ps_pro,
        ):
            # --- static SBUF tensors ---
            x_sb = singles.tile([128, Q // 128, C], F32, name="x_sb")
            xTq = [
                [singles.tile([128, 1024], BF16, name=f"xT{j}_{q}") for q in range(4)]
                for j in range(2)
            ]
            # f4: f^T tile i at partitions 32*(i%4), slot i//4
            f4 = singles.tile([128, MTILES // 4, 128], BF16, name="f4")
            # g4: g^T replicated at partition groups 0/32/64/96
            g4 = singles.tile([128, Q], BF16, name="g4")
            h_sb = singles.tile([CR, N], BF16, name="h_sb")
            h1_sb = singles.tile([128, MTILES, 33], BF16, name="h1_sb")
            wfgh_sb = singles.tile([128, 2, 96], BF16, name="wfgh_sb")
            bias4_sb = singles.tile([128, 3], F32, name="bias4_sb")
            wva2_sb = singles.tile([128, 128], BF16, name="wva2_sb")
            identb = singles.tile([32, 32], BF16, name="identb")
            identb128 = singles.tile([128, 128], BF16, name="identb128")
            ones128 = singles.tile([128, MTILES], F32, name="ones128")
            ones1b = singles.tile([33, 1], BF16, name="ones1b")
            dummy1 = singles.tile([1, 2], F32, name="dummy1")

            # --- local init first: the dummy activation makes walrus put
            # the ACT exp-table load at the head of the scalar queue,
            # before that queue's input DMAs ---
            nc.vector.memset(dummy1[:], 0.0)
            nc.scalar.activation(dummy1[:, 1:2], dummy1[:, 0:1], exp_fn)
            make_identity(nc, identb[:])
            make_identity(nc, identb128[:])
            nc.vector.memset(ones128[:], 1.0)
            nc.vector.memset(ones1b[:], 1.0)
            nc.vector.tensor_copy(h1_sb[:, :, 32], ones128[:])

            # --- input loads spread across the three DMA-capable queues ---
            nc.sync.dma_start(out=wfgh_sb[:], in_=wfgh_d)
            nc.scalar.dma_start(out=bias4_sb[:], in_=bias4_d)
            nc.sync.dma_start(out=xTq[0][0][:], in_=xt_d[0, :, 0:1024])
            nc.scalar.dma_start(out=xTq[1][0][:], in_=xt_d[1, :, 0:1024])
            nc.sync.dma_start(out=xTq[0][1][:], in_=xt_d[0, :, 1024:2048])
            nc.scalar.dma_start(out=xTq[1][1][:], in_=xt_d[1, :, 1024:2048])
            nc.sync.dma_start(out=xTq[0][2][:], in_=xt_d[0, :, 2048:3072])
            nc.gpsimd.dma_start(out=xTq[1][2][:], in_=xt_d[1, :, 2048:3072])
            nc.gpsimd.dma_start(out=xTq[0][3][:], in_=xt_d[0, :, 3072:4096])
            nc.gpsimd.dma_start(out=xTq[1][3][:], in_=xt_d[1, :, 3072:4096])
            nc.sync.dma_start(out=wva2_sb[:], in_=wva2_d)

            def produce_chunk(c):
                """f4/g4/h1 tiles for key rows [512c, 512c+512)."""
                nsl = slice(512 * c, 512 * (c + 1))
                pp = ps_pro.tile([96, 512], F32, tag="pro", name=f"pp{c}")
                qsl4 = slice(512 * (c % 2), 512 * (c % 2) + 512)
                nc.tensor.matmul(
                    pp[:], wfgh_sb[:, 0, :], xTq[0][c // 2][:, qsl4],
                    start=True, stop=False,
                )
                nc.tensor.matmul(
                    pp[:], wfgh_sb[:, 1, :], xTq[1][c // 2][:, qsl4],
                    start=False, stop=True,
                )
                # single staging copy frees the PSUM bank immediately.
                # chunks 0/1 skip it (next pp waits on its xt DMA anyway)
                # to keep the ~0.7us CAST off the startup critical path.
                # The copy runs on ACT: during qg0's m-loop the DVE is
                # saturated by the chunk drains while ACT has idle gaps.
                if c >= 2:
                    ppc = small.tile([96, 512], BF16, tag="ppc", name=f"ppc{c}")
                    nc.scalar.activation(ppc[:], pp[:], copy_fn)
                else:
                    ppc = pp
                for j in range(4):
                    psl = slice(32 * j, 32 * j + 32)
                    nc.vector.tensor_scalar_add(
                        f4[psl, c, :], ppc[0:32, 128 * j : 128 * j + 128],
                        bias4_sb[psl, 0:1],
                    )
                if c < 4:  # g only needed for this core's 2048 queries
                    nc.vector.tensor_scalar_add(
                        g4[0:32, nsl], ppc[32:64, :], bias4_sb[0:32, 1:2]
                    )
                    # replicate this query-chunk to partition groups 1-3.
                    # chunk 0 gates qg0's m-loop start: replicate on DVE
                    # (no DMA latency). Later chunks are needed only when
                    # their query group starts (~17us+ later): SBUF->SBUF
                    # DMAs on sync/gpsimd (NOT scalar: a blocked scalar
                    # dispatch would head-of-line block the exps).
                    if c == 0:
                        for j in range(1, 4):
                            nc.vector.tensor_copy(
                                g4[32 * j : 32 * j + 32, nsl], g4[0:32, nsl]
                            )
                    else:
                        nc.sync.dma_start(out=g4[32:64, nsl], in_=g4[0:32, nsl])
                        nc.sync.dma_start(out=g4[64:96, nsl], in_=g4[0:32, nsl])
                        nc.gpsimd.dma_start(out=g4[96:128, nsl], in_=g4[0:32, nsl])
                nc.vector.tensor_scalar_add(
                    h_sb[:, nsl], ppc[64:96, :], bias4_sb[0:32, 2:3]
                )
                # h1 tiles 4c..4c+3: h^T [32, 128] -> [128, 32], ones col stays
                ph = ps_pro.tile([128, 128], BF16, tag="pro", name=f"ph{c}")
                for k in range(4):
                    i = 4 * c + k
                    nc.tensor.matmul(
                        ph[:, 32 * k : 32 * k + 32],
                        h_sb[:, 128 * i : 128 * i + 128],
                        identb[:],
                        is_transpose=True,
                    )
                nc.vector.tensor_copy(h1_sb[:, 4 * c : 4 * c + 4, 0:32], ph[:])

            # --- prologue: first two chunks only; chunks 2-7 interleave
            # into qg0's m-loop (each emitted >=2 groups before its f4
            # consumer) so the PE queue starts attention work early ---
            PSCHED = {0: [2], 2: [3], 3: [4], 4: [5], 6: [6], 7: [7]}
            for c in (0, 1):
                produce_chunk(c)
            # residual x, needed only at the epilogues (t >= ~30us)
            x_view = x_d.rearrange("(t p) c -> p t c", p=128)
            for gq in range(2):
                nc.gpsimd.dma_start(
                    out=x_sb[:, 8 * gq : 8 * gq + 8, :],
                    in_=x_view[:, 8 * gq : 8 * gq + 8, :],
                )

            # --- m-loop machinery ---
            def mm1_group(qg, gi):
                grp = GROUPS[gi]
                qsl = slice(QG * qg, QG * qg + QG)
                sT = ps_sT.tile(
                    [128, len(grp), QG], F32, tag="sT", name=f"sT{qg}_{gi}"
                )
                for k, i in enumerate(grp):
                    j = i % 4
                    psl = slice(32 * j, 32 * j + 32)
                    nc.tensor.matmul(
                        sT[:, k, :],
                        f4[psl, i // 4, :],
                        g4[psl, qsl],
                        tile_position=(32 * j, 0),
                    )
                expt = expp.tile(
                    [128, len(grp), QG], BF16, tag="expt", name=f"expt{qg}_{gi}"
                )
                nc.scalar.activation(expt[:], sT[:], exp_fn)
                return expt

            def mm2_pair(qg, p, exptiles, acc):
                # acc is pre-zeroed by a DVE memset; start=False everywhere
                # makes the per-element has_written semantics correct no
                # matter how the bits are staled across the two col groups
                # (accumulate-onto-0 and overwrite are then equivalent).
                for col, i in enumerate((2 * p, 2 * p + 1)):
                    et, slot = exptiles[i]
                    base = 64 * col
                    nc.tensor.matmul(
                        acc[base : base + 33, :],
                        h1_sb[:, i, :],
                        et[:, slot, :],
                        start=False,
                        stop=(p == NPAIRS - 1 and col == 1),
                        skip_group_check=True,
                    )

            def epilogue_steps(qg, acc):
                """Yields (kind, fn) steps; kind 'pe' steps cost PE time.
                tail=True spreads work over ACT/GpSimd (idle in the tail)."""
                tail = qg == NQG - 1
                vv2 = small.tile([128, QG], BF16, tag="vv", name=f"vv{qg}")
                vhi = small.tile([33, QG], BF16, tag="vhi", name=f"vhi{qg}")
                rcp = small.tile([128, QG // 128], F32, tag="rcp", name=f"rcp{qg}")
                oT_sb = small.tile([128, 2, QG], BF16, tag="oTsb", name=f"oTsb{qg}")

                def s_vv():
                    if tail:  # split halves across ACT and DVE
                        nc.scalar.activation(vhi[:, 0:256], acc[64:97, 0:256], copy_fn)
                        nc.vector.tensor_copy(vhi[:, 256:512], acc[64:97, 256:512])
                    else:
                        nc.vector.tensor_copy(vhi[:], acc[64:97, :])
                    nc.vector.tensor_add(vv2[0:33, :], acc[0:33, :], vhi[:])
                    nc.vector.tensor_copy(vv2[64:97, :], vv2[0:33, :])

                def s_oT():
                    oT_ps = ps_sT.tile([128, 2, QG], F32, tag="sT", name=f"oT{qg}")
                    nc.tensor.matmul(oT_ps[:, 0, :], wva2_sb[0:33, :], vv2[0:33, :])
                    nc.tensor.matmul(oT_ps[:, 1, :], wva2_sb[64:97, :], vv2[64:97, :])
                    # denominator row -> per-partition layout via PE transpose
                    # bf16 PSUM writes must be 4B aligned: use stride-2 cols
                    dn_ps = ps_pro.tile(
                        [128, QG // 128, 2], BF16, tag="pro", name=f"dn{qg}"
                    )
                    for t in range(QG // 128):
                        nc.tensor.matmul(
                            dn_ps[:, t, 0:1],
                            vv2[32:33, 128 * t : 128 * t + 128],
                            ones1b[32:33, :],
                            is_transpose=True,
                        )
                    nc.vector.reciprocal(rcp[:], dn_ps[:, :, 0])
                    if tail:  # split ACT/DVE so po(t=0) starts early
                        nc.scalar.activation(
                            oT_sb[:, :, 0:256], oT_ps[:, :, 0:256], copy_fn
                        )
                        nc.vector.tensor_copy(
                            oT_sb[:, :, 256:512], oT_ps[:, :, 256:512]
                        )
                    else:
                        nc.vector.tensor_copy(oT_sb[:], oT_ps[:])

                yield "dve", s_vv
                yield "pe", s_oT

                def make_t(t):
                    def s_t():
                        # tail: po double-buffers in the (free) sT ring
                        po = (ps_sT if tail else ps_pro).tile(
                            [128, C], BF16, tag="sT" if tail else "pro",
                            name=f"po{qg}_{t}",
                        )
                        for half in range(2):
                            nc.tensor.matmul(
                                po[:, 128 * half : 128 * half + 128],
                                oT_sb[:, half, 128 * t : 128 * t + 128],
                                identb128[:],
                                is_transpose=True,
                            )
                        om = outp.tile([128, C], F32, tag="om", name=f"om{qg}_{t}")
                        outt = outp.tile([128, C], F32, tag="outt", name=f"outt{qg}_{t}")
                        if tail:
                            nc.scalar.activation(
                                om[:], po[:], copy_fn, scale=rcp[:, t : t + 1]
                            )
                        else:
                            nc.vector.tensor_scalar_mul(om[:], po[:], rcp[:, t : t + 1])
                        nc.vector.tensor_add(
                            outt[:], om[:], x_sb[:, (QG // 128) * qg + t, :]
                        )
                        row0 = QG * qg + 128 * t
                        queue = nc.scalar if (tail and t % 2) else nc.sync
                        queue.dma_start(out=out_d[row0 : row0 + 128, :], in_=outt[:])
                    return s_t

                for t in range(QG // 128):
                    yield "pe", make_t(t)

            # --- query-group loops with epilogue interleaving. The last
            # MM2 pairs of a query group are deferred until after the next
            # group's first MM1 so they don't head-block the PE queue
            # while waiting on the final exp. ---
            prev_epilogue = None  # generator of previous qg's epilogue steps
            carry_pairs = None    # deferred last pairs of the previous qg
            for qg in range(NQG):
                acc = None  # allocated after the previous qg's carried pairs
                exptiles = []
                next_pair = 0

                def drain_epilogue(n_pe):
                    nonlocal prev_epilogue
                    if prev_epilogue is None:
                        return
                    done = 0
                    for kind, fn in prev_epilogue:
                        fn()
                        if kind == "pe":
                            done += 1
                            if done >= n_pe:
                                return
                    prev_epilogue = None

                for gi, grp in enumerate(GROUPS):
                    if qg == 0:
                        for c in PSCHED.get(gi, ()):
                            produce_chunk(c)
                    expt = mm1_group(qg, gi)
                    for k in range(len(grp)):
                        exptiles.append((expt, k))
                    if gi == 1:
                        # carried pairs wait on the previous qg's last exp,
                        # which ran during this qg's first two MM1 groups
                        if carry_pairs is not None:
                            carry_pairs()
                            carry_pairs = None
                        # the previous epilogue's acc reads (s_vv) MUST be
                        # emitted before this qg's acc takes over the bank
                        if prev_epilogue is not None:
                            kind, fn = next(prev_epilogue)
                            assert kind == "dve"
                            fn()
                        acc = ps_acc.tile(
                            [128, QG], F32, tag="acc", name=f"acc{qg}"
                        )
                        nc.vector.memset(acc[:], 0.0)
                    # one epilogue PE step per group, starting once the
                    # carried-pair/epilogue dependency chain has cleared
                    if gi >= 3:
                        drain_epilogue(1)
                    # emit MM2 pairs whose tiles are in groups <= gi-1
                    if gi >= 2:
                        hi = 3 * gi - 1  # last tile index of group gi-1
                        while next_pair < NPAIRS and 2 * next_pair + 1 <= hi:
                            mm2_pair(qg, next_pair, exptiles, acc)
                            next_pair += 1
                drain_epilogue(99)
                if qg < NQG - 1:
                    def make_carry(qg_, tiles_, start_, acc_):
                        def emit():
                            p = start_
                            while p < NPAIRS:
                                mm2_pair(qg_, p, tiles_, acc_)
                                p += 1
                        return emit
                    carry_pairs = make_carry(qg, list(exptiles), next_pair, acc)
                else:
                    while next_pair < NPAIRS:
                        mm2_pair(qg, next_pair, exptiles, acc)
                        next_pair += 1
                prev_epilogue = epilogue_steps(qg, acc)
            # final epilogue (tail)
            for kind, fn in prev_epilogue:
                fn()

    return nc


_NC = None


def _get_nc():
    global _NC
    if _NC is None:
        _NC = build_program()
    return _NC


def _host_prep(inputs, Wf, bf, Wg, bg, Wh, bh, Wv, bv, gamma):
    import ml_dtypes

    x = np.asarray(inputs, np.float32).reshape(4, N, C)
    wfgh = np.concatenate(
        [np.asarray(Wf, np.float32), np.asarray(Wg, np.float32), np.asarray(Wh, np.float32)],
        axis=1,
    ).astype(ml_dtypes.bfloat16)  # [256, 96]
    # pre-arranged for a contiguous [128, 2, 96] load: [p, j, :] = row 128j+p
    wfgh = np.ascontiguousarray(wfgh.reshape(2, 128, 96).transpose(1, 0, 2))
    bias4 = np.zeros((128, 3), np.float32)
    bias4[:, 0] = np.tile(np.asarray(bf, np.float32), 4)
    bias4[:, 1] = np.tile(np.asarray(bg, np.float32), 4)
    bias4[0:32, 2] = np.asarray(bh, np.float32)
    gma = np.float32(np.asarray(gamma).reshape(-1)[0])
    wva = np.concatenate(
        [np.asarray(Wv, np.float32) * gma, (np.asarray(bv, np.float32) * gma)[None, :]],
        axis=0,
    )  # [33, 256] = [gamma*Wv; gamma*bv]
    wva2 = np.zeros((128, 128), np.float32)
    wva2[0:33, :] = wva[:, 0:128]
    wva2[64:97, :] = wva[:, 128:256]
    wva2 = wva2.astype(ml_dtypes.bfloat16)
    in_maps = []
    for core in range(8):
        b, qh = divmod(core, 2)
        xb = x[b]
        if qh:
            xb = np.roll(xb, -qh * Q, axis=0)
        xt = np.ascontiguousarray(
            xb.T.reshape(2, 128, N).astype(ml_dtypes.bfloat16)
        )  # [2, 128, N]: xt[j, p, n] = xb[n, 128j+p]
        in_maps.append(
            {
                "x": np.ascontiguousarray(xb[:Q]),
                "xt": xt,
                "wfgh": wfgh,
                "bias4": bias4,
                "wva2": wva2,
            }
        )
    return in_maps


def _gather(results, inputs_shape, dtype):
    out = np.empty((4, N, C), np.float32)
    for core in range(8):
        b, qh = divmod(core, 2)
        out[b, qh * Q : (qh + 1) * Q, :] = results[core]["out"]
    return out.reshape(inputs_shape).astype(dtype, copy=False)


def kernel(**inputs):
    from concourse.bass_utils import run_bass_kernel_spmd

    in_maps = _host_prep(**inputs)
    nc = _get_nc()
    res = run_bass_kernel_spmd(nc, in_maps, list(range(8)))
    x_in = np.asarray(inputs["inputs"])
    return _gather(res.results, x_in.shape, x_in.dtype)


def kernel_profiled(**inputs):
    """Like kernel() but with NTFF tracing; returns (out, BassKernelResults)."""
    import types

    if "antenv.axon_hooks" not in sys.modules:
        mod = types.ModuleType("antenv.axon_hooks")
        mod._h = None
        mod.set_axon_ntff_profile_hook = lambda h: setattr(mod, "_h", h)
        mod.get_axon_ntff_profile_hook = lambda: mod._h
        sys.modules["antenv.axon_hooks"] = mod
        try:
            from trn_agent_boot.trn_boot import _ntff_profile_via_ctypes

            mod._h = _ntff_profile_via_ctypes("/opt/axon/libaxon_pjrt.so")
        except Exception as e:  # profiling unavailable; run untraced
            print("NTFF hook unavailable:", e)
    from concourse.bass_utils import run_bass_kernel_spmd

    in_maps = _host_prep(**inputs)
    nc = _get_nc()
    res = run_bass_kernel_spmd(nc, in_maps, list(range(8)), trace=True)
    x_in = np.asarray(inputs["inputs"])
    return _gather(res.results, x_in.shape, x_in.dtype), res
